# revision 15
# baseline (speedup 1.0000x reference)
"""Trainium2 Bass kernel for nn_Conv2d_uint8_custom (dynamic uint8 quant + LUT conv).

Semantics implemented (matches reference.py):
  qf = clip(round(x/scale_f) + zero_f, 0, 255)          (per-tensor dynamic quant)
  qw = clip(round(w/scale_w) + zero_w, 0, 255)
  acc[b,o,l] = sum_k lut[qf_patch, qw] = sum_k qf*qw     (lut is an exact product table)
  out = (acc - zero_f * qw_sum[o]) * scale_f * scale_w + bias[o]

v2 strategy (vs the 43.5us baseline):
  * batch-parallel across 8 cores (2 images per core)
  * integer GEMM on TensorE in bf16 (ints 0..255 exact in bf16, fp32 PSUM -> bit-exact)
  * the row-shifted second copy of each feature (packing kh=0+kh=1 taps into K=128)
    is written by COMPUTE engines with a partition-crossing AP instead of an
    SBUF->SBUF DMA: removes the ~3us DMA-completion latency from the critical path
  * ~14 warm-up matmuls on scratch data ramp the PE HAM clock gate (1.2 -> 2.4 GHz)
    before the real GEMM starts, removing the half-rate cold phase
  * output written as bf16 (halves out HBM traffic; rel-err budget 2e-2 >> bf16 noise),
    host converts back to fp32
  * fewer tiles/instructions (single xs tile, one wq DMA, merged memsets) to shrink
    the Tile-framework event/release teardown tail
  * quant scalar constants baked as immediates; NEFF memoized on them
"""

import os
from contextlib import ExitStack

import numpy as np
import ml_dtypes

import concourse.bass as bass
import concourse.tile as tile
from concourse import bacc, mybir


def _ensure_axon_ntff_hook():
    """This image's `antenv` lacks `axon_hooks`, which bass_utils imports
    unconditionally when tracing under axon. Provide it (backed by the ctypes
    NTFF hook from trn_agent_boot when available, else None so concourse
    degrades to an untraced run)."""
    import sys, types

    if "antenv.axon_hooks" in sys.modules:
        return
    try:
        import antenv
    except ImportError:
        return
    mod = types.ModuleType("antenv.axon_hooks")
    hook = [None]
    try:
        from trn_agent_boot.trn_boot import _ntff_profile_via_ctypes

        hook[0] = _ntff_profile_via_ctypes("/opt/axon/libaxon_pjrt.so")
    except Exception:
        pass
    mod.get_axon_ntff_profile_hook = lambda: hook[0]
    mod.set_axon_ntff_profile_hook = lambda h: hook.__setitem__(0, h)
    sys.modules["antenv.axon_hooks"] = mod
    antenv.axon_hooks = mod


_ensure_axon_ntff_hook()

N_CORES = 8
B, C, H, W = 16, 64, 56, 56
O = 128
IMG_PER_CORE = B // N_CORES  # 2
L = H * W                    # 3136
HP, WP = H + 2, W + 2        # 58, 58 (zero-padded layout)
LP = HP * WP                 # 3364
TILE_ROWS = 8
NT = H // TILE_ROWS          # 7 output tiles per image
NCOL = TILE_ROWS * W         # 448 columns per tile (one PSUM bank)
MAGIC = 12582912.0           # 1.5 * 2**23: fp32 RNE integer-round trick
N_CHUNK = 4                  # quantization pipeline chunks (14 image rows each)
CH_ROWS = H // N_CHUNK       # 7
CH_COLS = CH_ROWS * W        # 392
N_WARM = 20                  # PE HAM warm-up matmuls (bridge until real MMs start)

FP32 = mybir.dt.float32
BF16 = mybir.dt.bfloat16

_CACHE = {}


def _build_nc(inv_s, sub_c, clip_hi, clip_lo, need_clip):
    """inv_s, sub_c (= MAGIC - zero_f), clip_hi/lo (magic-space clip bounds)
    are baked immediates. need_clip adds the min/max stage."""
    nc = bacc.Bacc(
        "TRN2",
        debug=False,
        enable_asserts=False,
        num_devices=N_CORES,
        enable_partition_id=False,
    )
    xs_d = nc.dram_tensor("xs", [2 * C, L], FP32, kind="ExternalInput").ap()
    wq_d = nc.dram_tensor("wq", [128, 6, 128], BF16, kind="ExternalInput").ap()
    qp_d = nc.dram_tensor("qp", [128, 2], FP32, kind="ExternalInput").ap()
    out_d = nc.dram_tensor(
        "out", [IMG_PER_CORE, O, L], BF16, kind="ExternalOutput"
    ).ap()

    with tile.TileContext(nc) as tc:
        with ExitStack() as ctx:
            _body(ctx, tc, xs_d, wq_d, qp_d, out_d, inv_s, sub_c, clip_hi,
                  clip_lo, need_clip)
    nc.compile()
    return nc


def _body(ctx, tc, xs_d, wq_d, qp_d, out_d, inv_s, sub_c, clip_hi, clip_lo,
          need_clip):
    nc = tc.nc
    A = mybir.AluOpType
    ID = mybir.ActivationFunctionType.Identity
    CP = mybir.ActivationFunctionType.Copy
    tpool = ctx.enter_context(tc.tile_pool(name="tmp", bufs=3))
    opool = ctx.enter_context(tc.tile_pool(name="osb", bufs=3))
    ppool = ctx.enter_context(tc.tile_pool(name="acc", bufs=7, space="PSUM"))
    wpool = ctx.enter_context(tc.tile_pool(name="wps", bufs=1, space="PSUM"))

    # write-once tensors live outside the tile pools: the Tile framework then
    # emits no end-of-program release instructions for them. releases cost
    # ~110ns of engine dispatch each and these tensors have the most reader
    # instructions (F0/F1: 84 matmuls, xs: 24 quant ops, wq: 84 ldweights).
    raw = lambda name, shape, dt: ctx.enter_context(
        nc.sbuf_tensor(name, shape, dt)
    )
    xs = raw("xs_sb", [128, L], FP32)
    wq = raw("wq_sb", [128, 6, 128], BF16)
    qp = raw("qp_sb", [128, 2], FP32)
    scr = raw("scr_sb", [128, NCOL], BF16)
    F0 = raw("F0_sb", [128, LP], BF16)
    F1 = raw("F1_sb", [128, LP], BF16)

    # warm-up scratch memset first so the PE warm-up chain starts immediately
    nc.gpsimd.memset(scr[:], 0.0)

    # input: one resident tensor, one column-sliced DMA per quant chunk (sync)
    for k in range(N_CHUNK):
        sl = slice(k * CH_COLS, (k + 1) * CH_COLS)
        nc.sync.dma_start(xs[:, sl], xs_d[:, sl])

    # weights + epilogue consts on the scalar ring (off the input ring)
    nc.scalar.dma_start(wq[:], wq_d[:])
    nc.scalar.dma_start(qp[:], qp_d[:])

    # F[img]: [128, 58, 58] bf16 padded quantized feature. partitions 0..63 =
    # image channels (rows 0..57), partitions 64..127 = same shifted up one
    # padded row, so a K=128 matmul covers taps kh=0 + kh=1, and the kh=2 tap
    # rides the 0..63 half with zeroed weights on 64..127.
    F0v = F0[:].rearrange("p (r c) -> p r c", c=WP)
    F1v = F1[:].rearrange("p (r c) -> p r c", c=WP)

    # zero borders (uint8 pad value 0). rows 56/57 of the shifted half and
    # row 57 / row 0 / side columns of the unshifted half. memsets that also
    # cover regions later overwritten by quant are harmless (WAW ordered).
    for v in (F0v, F1v):
        nc.gpsimd.memset(v[:, 0, :], 0.0)          # top pad row (both halves)
        nc.gpsimd.memset(v[:, 56:58, :], 0.0)      # bottom pad rows
        nc.gpsimd.memset(v[:, :, 0:1], 0.0)        # left pad col
        nc.gpsimd.memset(v[:, :, WP - 1 : WP], 0.0)  # right pad col

    # PE HAM warm-up: dummy matmuls on zeroed scratch keep the PE busy from
    # ~t0 so the clock gate opens (1.2 -> 2.4 GHz) before the real GEMM.
    warm = wpool.tile([128, NCOL], FP32, name="warm", tag="warm")
    for i in range(N_WARM):
        nc.tensor.matmul(
            warm[:], scr[:, 0:128], scr[:], start=True, stop=True,
            skip_group_check=True,
        )

    # quantize: t = x*inv_s + MAGIC   (fp32; the add performs exact RNE round)
    #           [optional clip to magic-space bounds]
    #           q = t - (MAGIC - zero) -> uint8 value, cast bf16 on write
    # each chunk covers 7 image rows; 4 strided writes land the value in both
    # the unshifted (rows r0..r0+6) and shifted (rows r0-1..r0+5) halves of
    # F0/F1 via partition-crossing APs -- no SBUF->SBUF DMA anywhere.
    def quant_chunk(ch):
        t1 = tpool.tile([128, CH_COLS], FP32, name="t1")
        nc.vector.tensor_scalar(
            t1[:], xs[:, ch * CH_COLS : (ch + 1) * CH_COLS], inv_s, MAGIC,
            op0=A.mult, op1=A.add,
        )
        src = t1
        if need_clip:
            c1 = tpool.tile([128, CH_COLS], FP32, name="c1")
            nc.vector.tensor_scalar(
                c1[:], t1[:], clip_hi, clip_lo, op0=A.min, op1=A.max
            )
            src = c1
        sv = src[:].rearrange("p (r c) -> p r c", c=W)
        r0 = 1 + ch * CH_ROWS
        # img0 on DVE, img1 on ACT (GPSIMD is ~16x slower for these shapes).
        # the shifted half is a cheap bf16->bf16 copy of the freshly-written
        # unshifted rows (same engine -> ordered, no cross-engine event).
        nc.vector.tensor_scalar(
            F0v[0:64, r0 : r0 + CH_ROWS, 1 : 1 + W], sv[0:64], sub_c, None,
            op0=A.subtract,
        )
        nc.vector.tensor_copy(
            F0v[64:128, r0 - 1 : r0 - 1 + CH_ROWS, 1 : 1 + W],
            F0v[0:64, r0 : r0 + CH_ROWS, 1 : 1 + W],
        )
        nc.scalar.activation(
            F1v[0:64, r0 : r0 + CH_ROWS, 1 : 1 + W], sv[64:128], CP,
            bias=-float(sub_c), scale=1.0,
        )
        # F1's shifted half also on DVE (369ns vs 950ns on ACT); reads ACT's
        # freshly written unshifted rows, so img1's data completes earlier
        nc.vector.tensor_copy(
            F1v[64:128, r0 - 1 : r0 - 1 + CH_ROWS, 1 : 1 + W],
            F1v[0:64, r0 : r0 + CH_ROWS, 1 : 1 + W],
        )

    # GEMM: per image, 7 tiles of [128 oc, 448 px]; per tile 6 matmuls:
    # g=0..2: kw=g, taps kh=0+1 (K=128); g=3..5: kw=g-3, tap kh=2 (K=64 used).
    # emission interleaves quant chunks between tile-pairs so the in-order
    # DVE/ACT queues run epilogues (and thus output DMAs) as PSUM tiles retire
    # instead of queuing them all behind the quant stream. adjacent t-pairs per
    # image share one staging buffer -> one 896-col output DMA per pair, on
    # alternating rings (sync for img0, gpsimd for img1).
    quant_chunk(0)
    quant_chunk(1)
    osb = {}
    for t in range(NT):
        if t in (1, 3) and t // 2 + 2 < N_CHUNK:
            quant_chunk(t // 2 + 2)
        for img in range(IMG_PER_CORE):
            fv = F0v if img == 0 else F1v
            ps = ppool.tile([128, NCOL], FP32, name=f"ps{img}_{t}", tag="ps")
            for g in range(6):
                kw = g % 3
                rt = TILE_ROWS * t + (0 if g < 3 else 2)
                nc.tensor.matmul(
                    ps[:],
                    wq[:, g, :],
                    fv[:, rt : rt + TILE_ROWS, kw : kw + W],
                    start=(g == 0),
                    stop=(g == 5),
                    skip_group_check=True,
                )
            if t % 2 == 0:
                osb[img] = opool.tile([128, 2 * NCOL], BF16, name=f"osb{img}")
            half = osb[img][:, (t % 2) * NCOL : (t % 2 + 1) * NCOL]
            if t == NT - 1:
                # final tile: split the epilogue across both engines so the
                # closing DMA issues as early as possible
                hw = NCOL // 2
                nc.scalar.activation(
                    half[:, 0:hw], ps[:, 0:hw], ID, bias=qp[:, 0:1],
                    scale=qp[:, 1:2],
                )
                nc.vector.tensor_scalar(
                    half[:, hw:NCOL], ps[:, hw:NCOL], qp[:, 1:2], qp[:, 0:1],
                    op0=A.mult, op1=A.add,
                )
            elif img == 0:
                nc.scalar.activation(
                    half, ps[:], ID, bias=qp[:, 0:1], scale=qp[:, 1:2]
                )
            else:
                nc.vector.tensor_scalar(
                    half, ps[:], qp[:, 1:2], qp[:, 0:1], op0=A.mult, op1=A.add
                )
            if t % 2 == 1 or t == NT - 1:
                lo = (t // 2) * 2 * NCOL
                width = NCOL if t == NT - 1 and t % 2 == 0 else 2 * NCOL
                ring = nc.sync if img == 0 else nc.gpsimd
                ring.dma_start(
                    out_d[img, :, lo : lo + width], osb[img][:, 0:width]
                )


def _quant_params_host(x, weight, bias):
    """Replicates the reference's fp32 quantization arithmetic bit-exactly
    (numpy and jax-on-cpu are both IEEE fp32, round-half-even)."""
    f = np.float32
    mx, mn = f(x.max()), f(x.min())
    scale_f = f((mx - mn) / f(255.0))
    zero_f = f(-np.round(mn / scale_f))
    inv_s = f(f(1.0) / scale_f)

    mw, nw = f(weight.max()), f(weight.min())
    scale_w = f((mw - nw) / f(255.0))
    zero_w = f(-np.round(nw / scale_w))
    qw = np.clip(
        np.round(weight.astype(np.float32) / scale_w) + zero_w, 0.0, 255.0
    ).astype(np.float32)  # exact small ints

    # exact emulation of the device quant to decide if clipping is ever live
    t = (x.astype(np.float32) * inv_s).astype(np.float32) + f(MAGIC)
    q_int = t.astype(np.float32) - f(MAGIC)
    need_clip = bool((q_int < -zero_f).any() or (q_int > f(255.0) - zero_f).any())

    s_tot = f(scale_f * scale_w)
    qw_sum = qw.reshape(O, -1).sum(axis=1, dtype=np.float64)
    bias_eff = (
        bias.astype(np.float64) - np.float64(zero_f) * qw_sum * np.float64(s_tot)
    ).astype(np.float32)

    qp = np.zeros((128, 2), np.float32)
    qp[:, 0] = bias_eff
    qp[:, 1] = s_tot

    consts = dict(
        inv_s=float(inv_s),
        sub_c=float(f(MAGIC) - zero_f),
        clip_hi=float(f(MAGIC) - zero_f + f(255.0)),
        clip_lo=float(f(MAGIC) - zero_f),
        need_clip=need_clip,
    )

    # weights [128 (K), 6 (g), 128 (O)] bf16; same for both images
    qwT = qw.transpose(2, 3, 1, 0)  # [kh, kw, C, O]
    wq = np.zeros((128, 6, 128), np.float32)
    for g in range(6):
        kw_ = g % 3
        if g < 3:
            wq[0:64, g] = qwT[0, kw_]
            wq[64:128, g] = qwT[1, kw_]
        else:
            wq[0:64, g] = qwT[2, kw_]
    return qp, wq.astype(ml_dtypes.bfloat16), consts


def build(consts=None):
    if consts is None:
        consts = dict(
            inv_s=1.0, sub_c=MAGIC - 127.0, clip_hi=MAGIC + 128.0,
            clip_lo=MAGIC - 127.0, need_clip=False,
        )
    key = tuple(sorted(consts.items()))
    if key not in _CACHE:
        _CACHE[key] = _build_nc(
            consts["inv_s"], consts["sub_c"], consts["clip_hi"],
            consts["clip_lo"], consts["need_clip"],
        )
    return _CACHE[key]


LAST_RESULT = None


def kernel(x, weight, bias, lut):
    global LAST_RESULT
    from concourse.bass_utils import run_bass_kernel_spmd

    x = np.asarray(x, dtype=np.float32)
    weight = np.asarray(weight, dtype=np.float32)
    bias = np.asarray(bias, dtype=np.float32)

    qp, wq, consts = _quant_params_host(x, weight, bias)
    nc = build(consts)
    in_maps = []
    for c in range(N_CORES):
        xs = np.ascontiguousarray(
            x[c * IMG_PER_CORE : (c + 1) * IMG_PER_CORE].reshape(2 * C, L)
        )
        in_maps.append({"xs": xs, "wq": wq, "qp": qp})

    res = run_bass_kernel_spmd(nc, in_maps, core_ids=list(range(N_CORES)))
    LAST_RESULT = res
    out = np.concatenate(
        [
            np.asarray(r["out"]).astype(np.float32).reshape(IMG_PER_CORE, O, H, W)
            for r in res.results
        ],
        axis=0,
    )
    return out


# revision 17
# speedup vs baseline: 1.2077x; 1.2077x over previous
"""Trainium2 Bass kernel for nn_Conv2d_uint8_custom (dynamic uint8 quant + LUT conv).

Semantics implemented (matches reference.py):
  qf = clip(round(x/scale_f) + zero_f, 0, 255)          (per-tensor dynamic quant)
  qw = clip(round(w/scale_w) + zero_w, 0, 255)
  acc[b,o,l] = sum_k lut[qf_patch, qw] = sum_k qf*qw     (lut is an exact product table)
  out = (acc - zero_f * qw_sum[o]) * scale_f * scale_w + bias[o]

v2 strategy (vs the 43.5us baseline):
  * batch-parallel across 8 cores (2 images per core)
  * integer GEMM on TensorE in bf16 (ints 0..255 exact in bf16, fp32 PSUM -> bit-exact)
  * the row-shifted second copy of each feature (packing kh=0+kh=1 taps into K=128)
    is written by COMPUTE engines with a partition-crossing AP instead of an
    SBUF->SBUF DMA: removes the ~3us DMA-completion latency from the critical path
  * ~14 warm-up matmuls on scratch data ramp the PE HAM clock gate (1.2 -> 2.4 GHz)
    before the real GEMM starts, removing the half-rate cold phase
  * output written as bf16 (halves out HBM traffic; rel-err budget 2e-2 >> bf16 noise),
    host converts back to fp32
  * fewer tiles/instructions (single xs tile, one wq DMA, merged memsets) to shrink
    the Tile-framework event/release teardown tail
  * quant scalar constants baked as immediates; NEFF memoized on them
"""

import os
from contextlib import ExitStack

import numpy as np
import ml_dtypes

import concourse.bass as bass
import concourse.tile as tile
from concourse import bacc, mybir


def _ensure_axon_ntff_hook():
    """This image's `antenv` lacks `axon_hooks`, which bass_utils imports
    unconditionally when tracing under axon. Provide it (backed by the ctypes
    NTFF hook from trn_agent_boot when available, else None so concourse
    degrades to an untraced run)."""
    import sys, types

    if "antenv.axon_hooks" in sys.modules:
        return
    try:
        import antenv
    except ImportError:
        return
    mod = types.ModuleType("antenv.axon_hooks")
    hook = [None]
    try:
        from trn_agent_boot.trn_boot import _ntff_profile_via_ctypes

        hook[0] = _ntff_profile_via_ctypes("/opt/axon/libaxon_pjrt.so")
    except Exception:
        pass
    mod.get_axon_ntff_profile_hook = lambda: hook[0]
    mod.set_axon_ntff_profile_hook = lambda h: hook.__setitem__(0, h)
    sys.modules["antenv.axon_hooks"] = mod
    antenv.axon_hooks = mod


_ensure_axon_ntff_hook()

N_CORES = 8
B, C, H, W = 16, 64, 56, 56
O = 128
IMG_PER_CORE = B // N_CORES  # 2
L = H * W                    # 3136
HP, WP = H + 2, W + 2        # 58, 58 (zero-padded layout)
LP = HP * WP                 # 3364
TILE_ROWS = 8
NT = H // TILE_ROWS          # 7 output tiles per image
NCOL = TILE_ROWS * W         # 448 columns per tile (one PSUM bank)
MAGIC = 12582912.0           # 1.5 * 2**23: fp32 RNE integer-round trick
N_CHUNK = 4                  # quantization pipeline chunks (14 image rows each)
CH_ROWS = H // N_CHUNK       # 7
CH_COLS = CH_ROWS * W        # 392
N_WARM = 20                  # PE HAM warm-up matmuls (bridge until real MMs start)

FP32 = mybir.dt.float32
BF16 = mybir.dt.bfloat16

_CACHE = {}


def _build_nc(inv_s, sub_c, clip_hi, clip_lo, need_clip):
    """inv_s, sub_c (= MAGIC - zero_f), clip_hi/lo (magic-space clip bounds)
    are baked immediates. need_clip adds the min/max stage."""
    nc = bacc.Bacc(
        "TRN2",
        debug=False,
        enable_asserts=False,
        num_devices=N_CORES,
        enable_partition_id=False,
    )
    xs_d = nc.dram_tensor("xs", [2 * C, L], FP32, kind="ExternalInput").ap()
    wq_d = nc.dram_tensor("wq", [128, 6, 128], BF16, kind="ExternalInput").ap()
    qp_d = nc.dram_tensor("qp", [128, 2], FP32, kind="ExternalInput").ap()
    out_d = nc.dram_tensor(
        "out", [IMG_PER_CORE, O, L], BF16, kind="ExternalOutput"
    ).ap()

    with tile.TileContext(nc) as tc:
        with ExitStack() as ctx:
            _body(ctx, tc, xs_d, wq_d, qp_d, out_d, inv_s, sub_c, clip_hi,
                  clip_lo, need_clip)
    nc.compile()
    return nc


def _body(ctx, tc, xs_d, wq_d, qp_d, out_d, inv_s, sub_c, clip_hi, clip_lo,
          need_clip):
    nc = tc.nc
    A = mybir.AluOpType
    ID = mybir.ActivationFunctionType.Identity
    CP = mybir.ActivationFunctionType.Copy
    tpool = ctx.enter_context(tc.tile_pool(name="tmp", bufs=3))
    opool = ctx.enter_context(tc.tile_pool(name="osb", bufs=3))
    ppool = ctx.enter_context(tc.tile_pool(name="acc", bufs=7, space="PSUM"))
    wpool = ctx.enter_context(tc.tile_pool(name="wps", bufs=1, space="PSUM"))

    # write-once tensors live outside the tile pools: the Tile framework then
    # emits no end-of-program release instructions for them. releases cost
    # ~110ns of engine dispatch each and these tensors have the most reader
    # instructions (F0/F1: 84 matmuls, xs: 24 quant ops, wq: 84 ldweights).
    raw = lambda name, shape, dt: ctx.enter_context(
        nc.sbuf_tensor(name, shape, dt)
    )
    xs = raw("xs_sb", [128, L], FP32)
    wq = raw("wq_sb", [128, 6, 128], BF16)
    qp = raw("qp_sb", [128, 2], FP32)
    scr = raw("scr_sb", [128, NCOL], BF16)
    F0 = raw("F0_sb", [128, LP], BF16)
    F1 = raw("F1_sb", [128, LP], BF16)

    # warm-up scratch memset first so the PE warm-up chain starts immediately
    nc.gpsimd.memset(scr[:], 0.0)

    # input: one resident tensor, one column-sliced DMA per quant chunk (sync)
    for k in range(N_CHUNK):
        sl = slice(k * CH_COLS, (k + 1) * CH_COLS)
        nc.sync.dma_start(xs[:, sl], xs_d[:, sl])

    # weights + epilogue consts on the scalar ring (off the input ring)
    nc.scalar.dma_start(wq[:], wq_d[:])
    nc.scalar.dma_start(qp[:], qp_d[:])

    # F[img]: [128, 58, 58] bf16 padded quantized feature. partitions 0..63 =
    # image channels (rows 0..57), partitions 64..127 = same shifted up one
    # padded row, so a K=128 matmul covers taps kh=0 + kh=1, and the kh=2 tap
    # rides the 0..63 half with zeroed weights on 64..127.
    F0v = F0[:].rearrange("p (r c) -> p r c", c=WP)
    F1v = F1[:].rearrange("p (r c) -> p r c", c=WP)

    # zero borders (uint8 pad value 0). rows 56/57 of the shifted half and
    # row 57 / row 0 / side columns of the unshifted half. memsets that also
    # cover regions later overwritten by quant are harmless (WAW ordered).
    for v in (F0v, F1v):
        nc.gpsimd.memset(v[:, 0, :], 0.0)          # top pad row (both halves)
        nc.gpsimd.memset(v[:, 56:58, :], 0.0)      # bottom pad rows
        nc.gpsimd.memset(v[:, :, 0:1], 0.0)        # left pad col
        nc.gpsimd.memset(v[:, :, WP - 1 : WP], 0.0)  # right pad col

    # PE HAM warm-up: dummy matmuls on zeroed scratch keep the PE busy from
    # ~t0 so the clock gate opens (1.2 -> 2.4 GHz) before the real GEMM.
    warm = wpool.tile([128, NCOL], FP32, name="warm", tag="warm")
    for i in range(N_WARM):
        nc.tensor.matmul(
            warm[:], scr[:, 0:128], scr[:], start=True, stop=True,
            skip_group_check=True,
        )

    # quantize: t = x*inv_s + MAGIC   (fp32; the add performs exact RNE round)
    #           [optional clip to magic-space bounds]
    #           q = t - (MAGIC - zero) -> uint8 value, cast bf16 on write
    # each chunk covers 7 image rows; 4 strided writes land the value in both
    # the unshifted (rows r0..r0+6) and shifted (rows r0-1..r0+5) halves of
    # F0/F1 via partition-crossing APs -- no SBUF->SBUF DMA anywhere.
    def quant_chunk(ch):
        t1 = tpool.tile([128, CH_COLS], FP32, name="t1")
        nc.vector.tensor_scalar(
            t1[:], xs[:, ch * CH_COLS : (ch + 1) * CH_COLS], inv_s, MAGIC,
            op0=A.mult, op1=A.add,
        )
        src = t1
        if need_clip:
            c1 = tpool.tile([128, CH_COLS], FP32, name="c1")
            nc.vector.tensor_scalar(
                c1[:], t1[:], clip_hi, clip_lo, op0=A.min, op1=A.max
            )
            src = c1
        sv = src[:].rearrange("p (r c) -> p r c", c=W)
        r0 = 1 + ch * CH_ROWS
        # img0 on DVE, img1 on ACT (GPSIMD is ~16x slower for these shapes).
        # the shifted half is a cheap bf16->bf16 copy of the freshly-written
        # unshifted rows (same engine -> ordered, no cross-engine event).
        nc.vector.tensor_scalar(
            F0v[0:64, r0 : r0 + CH_ROWS, 1 : 1 + W], sv[0:64], sub_c, None,
            op0=A.subtract,
        )
        nc.vector.tensor_copy(
            F0v[64:128, r0 - 1 : r0 - 1 + CH_ROWS, 1 : 1 + W],
            F0v[0:64, r0 : r0 + CH_ROWS, 1 : 1 + W],
        )
        nc.scalar.activation(
            F1v[0:64, r0 : r0 + CH_ROWS, 1 : 1 + W], sv[64:128], CP,
            bias=-float(sub_c), scale=1.0,
        )
        nc.scalar.copy(
            F1v[64:128, r0 - 1 : r0 - 1 + CH_ROWS, 1 : 1 + W],
            F1v[0:64, r0 : r0 + CH_ROWS, 1 : 1 + W],
        )

    # GEMM: per image, 7 tiles of [128 oc, 448 px]; per tile 6 matmuls:
    # g=0..2: kw=g, taps kh=0+1 (K=128); g=3..5: kw=g-3, tap kh=2 (K=64 used).
    # emission interleaves quant chunks between tile-pairs so the in-order
    # DVE/ACT queues run epilogues (and thus output DMAs) as PSUM tiles retire
    # instead of queuing them all behind the quant stream. adjacent t-pairs per
    # image share one staging buffer -> one 896-col output DMA per pair, on
    # alternating rings (sync for img0, gpsimd for img1).
    quant_chunk(0)
    quant_chunk(1)
    osb = {}
    for t in range(NT):
        if t in (1, 3) and t // 2 + 2 < N_CHUNK:
            quant_chunk(t // 2 + 2)
        for img in range(IMG_PER_CORE):
            fv = F0v if img == 0 else F1v
            ps = ppool.tile([128, NCOL], FP32, name=f"ps{img}_{t}", tag="ps")
            for g in range(6):
                kw = g % 3
                rt = TILE_ROWS * t + (0 if g < 3 else 2)
                nc.tensor.matmul(
                    ps[:],
                    wq[:, g, :],
                    fv[:, rt : rt + TILE_ROWS, kw : kw + W],
                    start=(g == 0),
                    stop=(g == 5),
                    skip_group_check=True,
                )
            if t % 2 == 0:
                osb[img] = opool.tile([128, 2 * NCOL], BF16, name=f"osb{img}")
            half = osb[img][:, (t % 2) * NCOL : (t % 2 + 1) * NCOL]
            if img == 0:
                nc.scalar.activation(
                    half, ps[:], ID, bias=qp[:, 0:1], scale=qp[:, 1:2]
                )
            else:
                nc.vector.tensor_scalar(
                    half, ps[:], qp[:, 1:2], qp[:, 0:1], op0=A.mult, op1=A.add
                )
            if t % 2 == 1 or t == NT - 1:
                lo = (t // 2) * 2 * NCOL
                width = NCOL if t == NT - 1 and t % 2 == 0 else 2 * NCOL
                ring = nc.sync if img == 0 else nc.gpsimd
                ring.dma_start(
                    out_d[img, :, lo : lo + width], osb[img][:, 0:width]
                )


def _quant_params_host(x, weight, bias):
    """Replicates the reference's fp32 quantization arithmetic bit-exactly
    (numpy and jax-on-cpu are both IEEE fp32, round-half-even)."""
    f = np.float32
    mx, mn = f(x.max()), f(x.min())
    scale_f = f((mx - mn) / f(255.0))
    zero_f = f(-np.round(mn / scale_f))
    inv_s = f(f(1.0) / scale_f)

    mw, nw = f(weight.max()), f(weight.min())
    scale_w = f((mw - nw) / f(255.0))
    zero_w = f(-np.round(nw / scale_w))
    qw = np.clip(
        np.round(weight.astype(np.float32) / scale_w) + zero_w, 0.0, 255.0
    ).astype(np.float32)  # exact small ints

    # exact emulation of the device quant to decide if clipping is ever live
    t = (x.astype(np.float32) * inv_s).astype(np.float32) + f(MAGIC)
    q_int = t.astype(np.float32) - f(MAGIC)
    need_clip = bool((q_int < -zero_f).any() or (q_int > f(255.0) - zero_f).any())

    s_tot = f(scale_f * scale_w)
    qw_sum = qw.reshape(O, -1).sum(axis=1, dtype=np.float64)
    bias_eff = (
        bias.astype(np.float64) - np.float64(zero_f) * qw_sum * np.float64(s_tot)
    ).astype(np.float32)

    qp = np.zeros((128, 2), np.float32)
    qp[:, 0] = bias_eff
    qp[:, 1] = s_tot

    consts = dict(
        inv_s=float(inv_s),
        sub_c=float(f(MAGIC) - zero_f),
        clip_hi=float(f(MAGIC) - zero_f + f(255.0)),
        clip_lo=float(f(MAGIC) - zero_f),
        need_clip=need_clip,
    )

    # weights [128 (K), 6 (g), 128 (O)] bf16; same for both images
    qwT = qw.transpose(2, 3, 1, 0)  # [kh, kw, C, O]
    wq = np.zeros((128, 6, 128), np.float32)
    for g in range(6):
        kw_ = g % 3
        if g < 3:
            wq[0:64, g] = qwT[0, kw_]
            wq[64:128, g] = qwT[1, kw_]
        else:
            wq[0:64, g] = qwT[2, kw_]
    return qp, wq.astype(ml_dtypes.bfloat16), consts


def build(consts=None):
    if consts is None:
        consts = dict(
            inv_s=1.0, sub_c=MAGIC - 127.0, clip_hi=MAGIC + 128.0,
            clip_lo=MAGIC - 127.0, need_clip=False,
        )
    key = tuple(sorted(consts.items()))
    if key not in _CACHE:
        _CACHE[key] = _build_nc(
            consts["inv_s"], consts["sub_c"], consts["clip_hi"],
            consts["clip_lo"], consts["need_clip"],
        )
    return _CACHE[key]


LAST_RESULT = None


def kernel(x, weight, bias, lut):
    global LAST_RESULT
    from concourse.bass_utils import run_bass_kernel_spmd

    x = np.asarray(x, dtype=np.float32)
    weight = np.asarray(weight, dtype=np.float32)
    bias = np.asarray(bias, dtype=np.float32)

    qp, wq, consts = _quant_params_host(x, weight, bias)
    nc = build(consts)
    in_maps = []
    for c in range(N_CORES):
        xs = np.ascontiguousarray(
            x[c * IMG_PER_CORE : (c + 1) * IMG_PER_CORE].reshape(2 * C, L)
        )
        in_maps.append({"xs": xs, "wq": wq, "qp": qp})

    res = run_bass_kernel_spmd(nc, in_maps, core_ids=list(range(N_CORES)))
    LAST_RESULT = res
    out = np.concatenate(
        [
            np.asarray(r["out"]).astype(np.float32).reshape(IMG_PER_CORE, O, H, W)
            for r in res.results
        ],
        axis=0,
    )
    return out
